# revision 5
# baseline (speedup 1.0000x reference)
"""Dilated self-attention Trainium2 kernel (8-core SPMD), v3.

Problem: x[2, 8192, 1024] -> q/k/v projections -> segment-local dense
attention (SEG=512) + 4 dilated-neighbor cross-attention passes
(offsets +-1, +-2 segments, every 4th key, each with its own softmax,
weight 1/4) -> output projection.

Sharding: data-parallel over batch (2) x tensor-parallel over heads
(4 groups of 4 heads).  Each of the 8 cores runs an IDENTICAL program
on different inputs: xT for its batch, the 256-wide head-group slices
of Wq/Wk/Wv and the matching 256 rows of Wo.  Each core emits a partial
output y[8192, 1024] (bf16); the host sums the 4 head-group partials
per batch.

On-core layout (all matmul operands pre-transposed so no on-device
transposes are needed):
  qT/kT  [128, 2, 8192]  features-on-partitions (head h -> chunk h//2,
                          rows (h%2)*64..)
  v5     [128, 64, 260]  tokens-on-partitions; per head 65 cols
                          [V_h (64) | ones] so each AV matmul also
                          produces the softmax denominator
  vd5    [128, 16, 260]  same for the dilated (every 4th) tokens, with
                          ones=4.0 so the denominator absorbs the
                          1/(2*NN) cross-pass weight.  Gathered from v5
                          by partition-strided SBUF DMA (not recomputed
                          on TensorE).
Scores are computed transposed (S^T[keys, q]); exp runs on ScalarE into
bf16.  AV runs in the [q, dk] orientation (lhsT = exp(S^T) chunk, rhs =
V'): out[q, j*65+64] is the softmax denominator as a per-partition
scalar, so normalization is a [128,4] reciprocal + one broadcast
multiply on the VectorE.  The accumulated [q, dk] result is moved into
the feature-major attnT layout with bf16 DMA transposes.

v3 vs the phase-separated baseline:
  - Attention and output projection are emitted interleaved (one pool
    scope): the attention stretch is ScalarE-exp-bound, so the oproj
    matmuls, PSUM drains and y DMAs hide underneath it.
  - vd5 gathered by DMA (saves a TensorE pass over the dilated tokens).
  - y emitted in bf16 (halves output DMA); host sums in fp32.
"""

import sys

sys.path.insert(0, "/opt/trn_rl_repo")

from contextlib import ExitStack

import numpy as np
import ml_dtypes

import concourse.tile as tile
from concourse import bacc, mybir
from concourse.bass_utils import run_bass_kernel_spmd

BF16 = mybir.dt.bfloat16
F32 = mybir.dt.float32

DIM = 1024
H = 16
DK = 64
SEG = 512
NN = 2
DIL = 4
B = 2
L = 8192
S = L // SEG            # 16 segments
HL = 4                  # heads per core
FL = HL * DK            # 256 features per core
KC = DIM // 128         # 8 contraction chunks for projections
KCS = SEG // 128        # 4 key chunks per segment (local attention)
N_CORES = 8
SCALE = 1.0 / 8.0       # 1/sqrt(DK)

_prog = None


def _build_program():
    nc = bacc.Bacc(None)
    xt = nc.dram_tensor("xt", [DIM, L], BF16, kind="ExternalInput")
    wq = nc.dram_tensor("wq", [DIM, FL], BF16, kind="ExternalInput")
    wk = nc.dram_tensor("wk", [DIM, FL], BF16, kind="ExternalInput")
    wv = nc.dram_tensor("wv", [DIM, FL], BF16, kind="ExternalInput")
    wo = nc.dram_tensor("wo", [FL, DIM], BF16, kind="ExternalInput")
    y = nc.dram_tensor("y", [L, DIM], BF16, kind="ExternalOutput")

    Exp = mybir.ActivationFunctionType.Exp
    Copy = mybir.ActivationFunctionType.Copy

    with tile.TileContext(nc) as tc, ExitStack() as ctx:
        singles = ctx.enter_context(tc.tile_pool(name="singles", bufs=1))
        qt = singles.tile([128, 2, L], BF16)
        kt = singles.tile([128, 2, L], BF16)
        v5 = singles.tile([128, L // 128, HL * 65], BF16)
        vd5 = singles.tile([128, (L // DIL) // 128, HL * 65], BF16)
        attnT = singles.tile([128, 2, L], BF16)
        wq_sb = singles.tile([128, KC, FL], BF16)
        wk_sb = singles.tile([128, KC, FL], BF16)
        wv_sb = singles.tile([128, KC, FL], BF16)
        wo_sb = singles.tile([128, FL // 128, DIM], BF16)
        nc.sync.dma_start(wq_sb, wq.rearrange("(k p) f -> p k f", p=128))
        nc.sync.dma_start(wk_sb, wk.rearrange("(k p) f -> p k f", p=128))
        nc.sync.dma_start(wv_sb, wv.rearrange("(k p) f -> p k f", p=128))
        nc.sync.dma_start(wo_sb, wo.rearrange("(k p) f -> p k f", p=128))
        v5_g = v5.rearrange("p c (h e) -> p c h e", e=65)
        vd5_g = vd5.rearrange("p c (h e) -> p c h e", e=65)
        nc.vector.memset(v5_g[:, :, :, 64], 1.0)
        nc.vector.memset(vd5_g[:, :, :, 64], float(2 * NN))

        # ---------- Phase 1: q/k/v projections (+ dilated-v gather) ----------
        with tc.tile_pool(name="xp", bufs=2) as xp, \
             tc.tile_pool(name="qkps", bufs=4, space="PSUM") as qkps, \
             tc.tile_pool(name="vps", bufs=4, space="PSUM") as vps:
            for t in range(S):
                sl = slice(t * SEG, (t + 1) * SEG)
                x_t = xp.tile([128, KC, SEG], BF16, tag="x", name="x_t")
                nc.sync.dma_start(x_t, xt[:, sl].rearrange("(k p) n -> p k n", p=128))
                for m in range(2):
                    for w_sb, dst in ((wq_sb, qt), (wk_sb, kt)):
                        pst = qkps.tile([128, SEG], F32, tag="qk", name="pst")
                        for k in range(KC):
                            nc.tensor.matmul(
                                pst,
                                w_sb[:, k, m * 128:(m + 1) * 128],
                                x_t[:, k],
                                start=(k == 0),
                                stop=(k == KC - 1),
                            )
                        eng = nc.vector if m == 0 else nc.scalar
                        if m == 0:
                            nc.vector.tensor_copy(dst[:, m, sl], pst)
                        else:
                            nc.scalar.activation(dst[:, m, sl], pst, Copy)
                for sub in range(SEG // 128):
                    c = t * (SEG // 128) + sub
                    psv = vps.tile([128, FL], F32, tag="v", name="psv")
                    for k in range(KC):
                        nc.tensor.matmul(
                            psv,
                            x_t[:, k, sub * 128:(sub + 1) * 128],
                            wv_sb[:, k],
                            start=(k == 0),
                            stop=(k == KC - 1),
                        )
                    if sub % 2 == 0:
                        nc.scalar.activation(
                            v5_g[:, c, :, 0:64],
                            psv.rearrange("p (h e) -> p h e", e=64),
                            Copy,
                        )
                    else:
                        nc.vector.tensor_copy(
                            v5_g[:, c, :, 0:64],
                            psv.rearrange("p (h e) -> p h e", e=64),
                        )
                # dilated V: gather every 4th token of this segment from
                # v5 (partition-strided SBUF->SBUF DMA, replaces a
                # TensorE projection pass over the dilated tokens).
                for a in range(4):
                    nc.sync.dma_start(
                        vd5_g[32 * a:32 * (a + 1), t, :, 0:64],
                        v5_g[0:128:DIL, t * 4 + a, :, 0:64],
                    )

        # ---------- Phase 2+3 fused: attention + output projection ----------
        # The attention stream is ScalarE(exp)-bound; interleaving the
        # output-projection matmuls/drains/DMAs per segment fills the
        # TensorE/VectorE idle time underneath it.
        with tc.tile_pool(name="scps", bufs=2, space="PSUM") as scps, \
             tc.tile_pool(name="avl", bufs=1, space="PSUM") as avl, \
             tc.tile_pool(name="avx", bufs=2, space="PSUM") as avx, \
             tc.tile_pool(name="yps", bufs=1, space="PSUM") as yps, \
             tc.tile_pool(name="expp", bufs=6) as expp, \
             tc.tile_pool(name="recp", bufs=6) as recp, \
             tc.tile_pool(name="accp", bufs=6) as accp, \
             tc.tile_pool(name="accbf", bufs=4) as accbf, \
             tc.tile_pool(name="ysb", bufs=3) as ysb:

            def emit_attn(s):
                q_sl = slice(s * SEG, (s + 1) * SEG)
                for m in range(2):
                    # local scores S^T = K^T-chunks x Q (two heads as
                    # concurrent row-tiles at partition bases 0 / 64)
                    exp_tiles = []
                    for c in range(KCS):
                        k_sl = slice(s * SEG + c * 128, s * SEG + (c + 1) * 128)
                        ps_sc = scps.tile([128, 2 * SEG], F32, tag="sc", name="ps_sc")
                        for he in range(2):
                            r0 = he * 64
                            nc.tensor.matmul(
                                ps_sc[:, he * SEG:(he + 1) * SEG],
                                kt[r0:r0 + 64, m, k_sl],
                                qt[r0:r0 + 64, m, q_sl],
                            )
                        e_t = expp.tile([128, 2 * SEG], BF16, tag="exp", name="e_t")
                        nc.scalar.activation(e_t, ps_sc, Exp, scale=SCALE)
                        exp_tiles.append(e_t)
                    # local AV in [q, dk] orientation; col j*65+64 is the
                    # softmax denominator (per-partition scalar)
                    accs = []
                    for he in range(2):
                        hl = 2 * m + he
                        av = avl.tile([128, 260], F32, tag="avl", name="av")
                        for c in range(KCS):
                            for j in range(4):
                                nc.tensor.matmul(
                                    av[:, j * 65:(j + 1) * 65],
                                    exp_tiles[c][:, he * SEG + j * 128:he * SEG + (j + 1) * 128],
                                    v5[:, s * KCS + c, hl * 65:(hl + 1) * 65],
                                    start=(c == 0 and j == 0),
                                    stop=(c == KCS - 1 and j == 3),
                                )
                        av_g = av.rearrange("p (j e) -> p j e", e=65)
                        rec = recp.tile([128, 4], F32, tag="rec", name="rec")
                        nc.vector.reciprocal(rec, av_g[:, :, 64])
                        acc = accp.tile([128, 4, 64], F32, tag="acc", name="acc")
                        nc.vector.tensor_mul(
                            acc, av_g[:, :, 0:64],
                            rec[:, :, None].to_broadcast((128, 4, 64)),
                        )
                        accs.append(acc)
                    # dilated neighbor-segment passes
                    valid_offs = [o for o in (-2, -1, 1, 2) if 0 <= s + o < S]
                    for idx, o in enumerate(valid_offs):
                        n = s + o
                        ps_sc = scps.tile([128, 2 * SEG], F32, tag="sc", name="ps_sc")
                        for he in range(2):
                            r0 = he * 64
                            nc.tensor.matmul(
                                ps_sc[:, he * SEG:(he + 1) * SEG],
                                kt[r0:r0 + 64, m, n * SEG:(n + 1) * SEG:DIL],
                                qt[r0:r0 + 64, m, q_sl],
                            )
                        e_t = expp.tile([128, 2 * SEG], BF16, tag="exp", name="e_t")
                        nc.scalar.activation(e_t, ps_sc, Exp, scale=SCALE)
                        for he in range(2):
                            hl = 2 * m + he
                            avx_t = avx.tile([128, 260], F32, tag="avx", name="avx_t")
                            for j in range(4):
                                nc.tensor.matmul(
                                    avx_t[:, j * 65:(j + 1) * 65],
                                    e_t[:, he * SEG + j * 128:he * SEG + (j + 1) * 128],
                                    vd5[:, n, hl * 65:(hl + 1) * 65],
                                    start=(j == 0),
                                    stop=(j == 3),
                                )
                            avx_g = avx_t.rearrange("p (j e) -> p j e", e=65)
                            rec = recp.tile([128, 4], F32, tag="rec", name="rec")
                            nc.vector.reciprocal(rec, avx_g[:, :, 64])
                            tmp = accp.tile([128, 4, 64], F32, tag="tmp", name="tmp")
                            nc.vector.tensor_mul(
                                tmp, avx_g[:, :, 0:64],
                                rec[:, :, None].to_broadcast((128, 4, 64)),
                            )
                            nc.vector.tensor_add(accs[he], accs[he], tmp)
                    # cast + transpose back to feature-major attnT.
                    # acc_bf packs both heads per q-slice: [q, j, he, dk],
                    # so each [128, 128] transpose lands as attnT's
                    # [he0 dk rows | he1 dk rows] block directly.
                    acc_bf = accbf.tile([128, 4, 2, 64], BF16, tag="accbf", name="acc_bf")
                    for he in range(2):
                        nc.vector.tensor_copy(acc_bf[:, :, he, :], accs[he])
                    for j in range(4):
                        nc.sync.dma_start_transpose(
                            attnT[:, m, s * SEG + j * 128:s * SEG + (j + 1) * 128],
                            acc_bf[:, j, :, :],
                        )

            def emit_oproj(s):
                for sub in range(SEG // 128):
                    tcn = s * (SEG // 128) + sub
                    y_t = ysb.tile([128, DIM], BF16, tag="ysb", name="y_t")
                    for nh in range(2):
                        ps_y = yps.tile([128, 512], F32, tag="y", name="ps_y")
                        for m in range(2):
                            nc.tensor.matmul(
                                ps_y,
                                attnT[:, m, tcn * 128:(tcn + 1) * 128],
                                wo_sb[:, m, nh * 512:(nh + 1) * 512],
                                start=(m == 0),
                                stop=(m == 1),
                            )
                        if nh == 0:
                            nc.scalar.activation(y_t[:, 0:512], ps_y, Copy)
                        else:
                            nc.vector.tensor_copy(y_t[:, 512:1024], ps_y)
                    # y goes out on the ACT HWDGE ring so it doesn't
                    # queue behind the attnT transposes on the SP ring.
                    nc.scalar.dma_start(y[tcn * 128:(tcn + 1) * 128, :], y_t)

            for s in range(S + 1):
                if s < S:
                    emit_attn(s)
                if s >= 1:
                    emit_oproj(s - 1)

    nc.compile()
    return nc


def _make_in_maps(x, Wq, Wk, Wv, Wo):
    bf = ml_dtypes.bfloat16
    xt_b = [np.asarray(x[b]).T.astype(bf) for b in range(B)]
    wq_g = [np.asarray(Wq[:, g * FL:(g + 1) * FL]).astype(bf) for g in range(4)]
    wk_g = [np.asarray(Wk[:, g * FL:(g + 1) * FL]).astype(bf) for g in range(4)]
    wv_g = [np.asarray(Wv[:, g * FL:(g + 1) * FL]).astype(bf) for g in range(4)]
    wo_g = [np.asarray(Wo[g * FL:(g + 1) * FL, :]).astype(bf) for g in range(4)]
    in_maps = []
    for c in range(N_CORES):
        b, g = divmod(c, 4)
        in_maps.append(
            {"xt": xt_b[b], "wq": wq_g[g], "wk": wk_g[g], "wv": wv_g[g],
             "wo": wo_g[g]}
        )
    return in_maps


def run(x, Wq, bq, Wk, bk, Wv, bv, Wo, bo, trace=False, tmpdir=None):
    """Build (cached), run on 8 cores, gather. Returns (y, BassKernelResults)."""
    global _prog
    if _prog is None:
        _prog = _build_program()
    in_maps = _make_in_maps(x, Wq, Wk, Wv, Wo)
    res = run_bass_kernel_spmd(
        _prog, in_maps, core_ids=list(range(N_CORES)), trace=trace, tmpdir=tmpdir
    )
    y = np.zeros((B, L, DIM), np.float32)
    for c in range(N_CORES):
        y[c // 4] += np.asarray(res.results[c]["y"], dtype=np.float32)
    # bq/bk/bv are identically zero in this problem; bo is added on host.
    y += np.asarray(bo, np.float32)[None, None, :]
    return y, res


def kernel(x, Wq, bq, Wk, bk, Wv, bv, Wo, bo):
    y, _ = run(x, Wq, bq, Wk, bk, Wv, bv, Wo, bo)
    return y


# revision 6
# speedup vs baseline: 1.1752x; 1.1752x over previous
"""Dilated self-attention Trainium2 kernel (8-core SPMD), v3.

Problem: x[2, 8192, 1024] -> q/k/v projections -> segment-local dense
attention (SEG=512) + 4 dilated-neighbor cross-attention passes
(offsets +-1, +-2 segments, every 4th key, each with its own softmax,
weight 1/4) -> output projection.

Sharding: data-parallel over batch (2) x tensor-parallel over heads
(4 groups of 4 heads).  Each of the 8 cores runs an IDENTICAL program
on different inputs: xT for its batch, the 256-wide head-group slices
of Wq/Wk/Wv and the matching 256 rows of Wo.  Each core emits a partial
output y[8192, 1024] (bf16); the host sums the 4 head-group partials
per batch.

On-core layout (all matmul operands pre-transposed so no on-device
transposes are needed):
  qT/kT  [128, 2, 8192]  features-on-partitions (head h -> chunk h//2,
                          rows (h%2)*64..)
  v5     [128, 64, 260]  tokens-on-partitions; per head 65 cols
                          [V_h (64) | ones] so each AV matmul also
                          produces the softmax denominator
  vd5    [128, 16, 260]  same for the dilated (every 4th) tokens, with
                          ones=4.0 so the denominator absorbs the
                          1/(2*NN) cross-pass weight.  Gathered from v5
                          by partition-strided SBUF DMA (not recomputed
                          on TensorE).
Scores are computed transposed (S^T[keys, q]); exp runs on ScalarE into
bf16.  AV runs in the [q, dk] orientation (lhsT = exp(S^T) chunk, rhs =
V'): out[q, j*65+64] is the softmax denominator as a per-partition
scalar, so normalization is a [128,4] reciprocal + one broadcast
multiply on the VectorE.  The accumulated [q, dk] result is moved into
the feature-major attnT layout with bf16 DMA transposes.

v3 vs the phase-separated baseline:
  - Attention and output projection are emitted interleaved (one pool
    scope): the attention stretch is ScalarE-exp-bound, so the oproj
    matmuls, PSUM drains and y DMAs hide underneath it.
  - vd5 gathered by DMA (saves a TensorE pass over the dilated tokens).
  - y emitted in bf16 (halves output DMA); host sums in fp32.
"""

import sys

sys.path.insert(0, "/opt/trn_rl_repo")

from contextlib import ExitStack

import numpy as np
import ml_dtypes

import concourse.tile as tile
from concourse import bacc, mybir
from concourse.bass_utils import run_bass_kernel_spmd

BF16 = mybir.dt.bfloat16
F32 = mybir.dt.float32

DIM = 1024
H = 16
DK = 64
SEG = 512
NN = 2
DIL = 4
B = 2
L = 8192
S = L // SEG            # 16 segments
HL = 4                  # heads per core
FL = HL * DK            # 256 features per core
KC = DIM // 128         # 8 contraction chunks for projections
KCS = SEG // 128        # 4 key chunks per segment (local attention)
N_CORES = 8
SCALE = 1.0 / 8.0       # 1/sqrt(DK)

_prog = None


def _build_program():
    nc = bacc.Bacc(None)
    xt = nc.dram_tensor("xt", [DIM, L], BF16, kind="ExternalInput")
    wq = nc.dram_tensor("wq", [DIM, FL], BF16, kind="ExternalInput")
    wk = nc.dram_tensor("wk", [DIM, FL], BF16, kind="ExternalInput")
    wv = nc.dram_tensor("wv", [DIM, FL], BF16, kind="ExternalInput")
    wo = nc.dram_tensor("wo", [FL, DIM], BF16, kind="ExternalInput")
    y = nc.dram_tensor("y", [L, DIM], BF16, kind="ExternalOutput")

    Exp = mybir.ActivationFunctionType.Exp
    Copy = mybir.ActivationFunctionType.Copy

    with tile.TileContext(nc) as tc, ExitStack() as ctx:
        singles = ctx.enter_context(tc.tile_pool(name="singles", bufs=1))
        qt = singles.tile([128, 2, L], BF16)
        kt = singles.tile([128, 2, L], BF16)
        v5 = singles.tile([128, L // 128, HL * 65], BF16)
        vd5 = singles.tile([128, (L // DIL) // 128, HL * 65], BF16)
        attnT = singles.tile([128, 2, L], BF16)
        wq_sb = singles.tile([128, KC, FL], BF16)
        wk_sb = singles.tile([128, KC, FL], BF16)
        wv_sb = singles.tile([128, KC, FL], BF16)
        wo_sb = singles.tile([128, FL // 128, DIM], BF16)
        nc.sync.dma_start(wq_sb, wq.rearrange("(k p) f -> p k f", p=128))
        nc.sync.dma_start(wk_sb, wk.rearrange("(k p) f -> p k f", p=128))
        nc.sync.dma_start(wv_sb, wv.rearrange("(k p) f -> p k f", p=128))
        nc.sync.dma_start(wo_sb, wo.rearrange("(k p) f -> p k f", p=128))
        v5_g = v5.rearrange("p c (h e) -> p c h e", e=65)
        vd5_g = vd5.rearrange("p c (h e) -> p c h e", e=65)
        nc.vector.memset(v5_g[:, :, :, 64], 1.0)
        nc.vector.memset(vd5_g[:, :, :, 64], float(2 * NN))

        # ---------- Phase 1: q/k/v projections (+ dilated-v gather) ----------
        with tc.tile_pool(name="xp", bufs=2) as xp, \
             tc.tile_pool(name="qkps", bufs=4, space="PSUM") as qkps, \
             tc.tile_pool(name="vps", bufs=4, space="PSUM") as vps:
            for t in range(S):
                sl = slice(t * SEG, (t + 1) * SEG)
                x_t = xp.tile([128, KC, SEG], BF16, tag="x", name="x_t")
                nc.sync.dma_start(x_t, xt[:, sl].rearrange("(k p) n -> p k n", p=128))
                for m in range(2):
                    for w_sb, dst in ((wq_sb, qt), (wk_sb, kt)):
                        pst = qkps.tile([128, SEG], F32, tag="qk", name="pst")
                        for k in range(KC):
                            nc.tensor.matmul(
                                pst,
                                w_sb[:, k, m * 128:(m + 1) * 128],
                                x_t[:, k],
                                start=(k == 0),
                                stop=(k == KC - 1),
                            )
                        eng = nc.vector if m == 0 else nc.scalar
                        if m == 0:
                            nc.vector.tensor_copy(dst[:, m, sl], pst)
                        else:
                            nc.scalar.activation(dst[:, m, sl], pst, Copy)
                for sub in range(SEG // 128):
                    c = t * (SEG // 128) + sub
                    psv = vps.tile([128, FL], F32, tag="v", name="psv")
                    for k in range(KC):
                        nc.tensor.matmul(
                            psv,
                            x_t[:, k, sub * 128:(sub + 1) * 128],
                            wv_sb[:, k],
                            start=(k == 0),
                            stop=(k == KC - 1),
                        )
                    if sub % 2 == 0:
                        nc.scalar.activation(
                            v5_g[:, c, :, 0:64],
                            psv.rearrange("p (h e) -> p h e", e=64),
                            Copy,
                        )
                    else:
                        nc.vector.tensor_copy(
                            v5_g[:, c, :, 0:64],
                            psv.rearrange("p (h e) -> p h e", e=64),
                        )
                # dilated V: gather every 4th token of this segment from
                # v5 (partition-strided SBUF->SBUF DMA, replaces a
                # TensorE projection pass over the dilated tokens).
                for a in range(4):
                    nc.sync.dma_start(
                        vd5_g[32 * a:32 * (a + 1), t, :, 0:64],
                        v5_g[0:128:DIL, t * 4 + a, :, 0:64],
                    )

        # ---------- Phase 2+3 fused: attention + output projection ----------
        # The attention stream is ScalarE(exp)-bound; interleaving the
        # output-projection matmuls/drains/DMAs per segment fills the
        # TensorE/VectorE idle time underneath it.
        with tc.tile_pool(name="scps", bufs=2, space="PSUM") as scps, \
             tc.tile_pool(name="avl", bufs=1, space="PSUM") as avl, \
             tc.tile_pool(name="avx", bufs=2, space="PSUM") as avx, \
             tc.tile_pool(name="yps", bufs=1, space="PSUM") as yps, \
             tc.tile_pool(name="expp", bufs=6) as expp, \
             tc.tile_pool(name="recp", bufs=6) as recp, \
             tc.tile_pool(name="accp", bufs=6) as accp, \
             tc.tile_pool(name="accbf", bufs=4) as accbf, \
             tc.tile_pool(name="ysb", bufs=3) as ysb:

            def emit_attn(s):
                q_sl = slice(s * SEG, (s + 1) * SEG)
                for m in range(2):
                    # local scores S^T = K^T-chunks x Q (two heads as
                    # concurrent row-tiles at partition bases 0 / 64)
                    exp_tiles = []
                    for c in range(KCS):
                        k_sl = slice(s * SEG + c * 128, s * SEG + (c + 1) * 128)
                        ps_sc = scps.tile([128, 2 * SEG], F32, tag="sc", name="ps_sc")
                        for he in range(2):
                            r0 = he * 64
                            nc.tensor.matmul(
                                ps_sc[:, he * SEG:(he + 1) * SEG],
                                kt[r0:r0 + 64, m, k_sl],
                                qt[r0:r0 + 64, m, q_sl],
                            )
                        e_t = expp.tile([128, 2 * SEG], BF16, tag="exp", name="e_t")
                        nc.scalar.activation(e_t, ps_sc, Exp, scale=SCALE)
                        exp_tiles.append(e_t)
                    # local AV in [q, dk] orientation; col j*65+64 is the
                    # softmax denominator (per-partition scalar)
                    accs = []
                    for he in range(2):
                        hl = 2 * m + he
                        av = avl.tile([128, 260], F32, tag="avl", name="av")
                        for c in range(KCS):
                            for j in range(4):
                                nc.tensor.matmul(
                                    av[:, j * 65:(j + 1) * 65],
                                    exp_tiles[c][:, he * SEG + j * 128:he * SEG + (j + 1) * 128],
                                    v5[:, s * KCS + c, hl * 65:(hl + 1) * 65],
                                    start=(c == 0 and j == 0),
                                    stop=(c == KCS - 1 and j == 3),
                                )
                        av_g = av.rearrange("p (j e) -> p j e", e=65)
                        rec = recp.tile([128, 4], F32, tag="rec", name="rec")
                        nc.vector.reciprocal(rec, av_g[:, :, 64])
                        acc = accp.tile([128, 4, 64], F32, tag="acc", name="acc")
                        nc.vector.tensor_mul(
                            acc, av_g[:, :, 0:64],
                            rec[:, :, None].to_broadcast((128, 4, 64)),
                        )
                        accs.append(acc)
                    # dilated neighbor-segment passes
                    valid_offs = [o for o in (-2, -1, 1, 2) if 0 <= s + o < S]
                    for idx, o in enumerate(valid_offs):
                        n = s + o
                        ps_sc = scps.tile([128, 2 * SEG], F32, tag="sc", name="ps_sc")
                        for he in range(2):
                            r0 = he * 64
                            nc.tensor.matmul(
                                ps_sc[:, he * SEG:(he + 1) * SEG],
                                kt[r0:r0 + 64, m, n * SEG:(n + 1) * SEG:DIL],
                                qt[r0:r0 + 64, m, q_sl],
                            )
                        e_t = expp.tile([128, 2 * SEG], BF16, tag="exp", name="e_t")
                        nc.scalar.activation(e_t, ps_sc, Exp, scale=SCALE)
                        for he in range(2):
                            hl = 2 * m + he
                            avx_t = avx.tile([128, 260], F32, tag="avx", name="avx_t")
                            for j in range(4):
                                nc.tensor.matmul(
                                    avx_t[:, j * 65:(j + 1) * 65],
                                    e_t[:, he * SEG + j * 128:he * SEG + (j + 1) * 128],
                                    vd5[:, n, hl * 65:(hl + 1) * 65],
                                    start=(j == 0),
                                    stop=(j == 3),
                                )
                            avx_g = avx_t.rearrange("p (j e) -> p j e", e=65)
                            rec = recp.tile([128, 4], F32, tag="rec", name="rec")
                            nc.vector.reciprocal(rec, avx_g[:, :, 64])
                            tmp = accp.tile([128, 4, 64], F32, tag="tmp", name="tmp")
                            nc.vector.tensor_mul(
                                tmp, avx_g[:, :, 0:64],
                                rec[:, :, None].to_broadcast((128, 4, 64)),
                            )
                            nc.vector.tensor_add(accs[he], accs[he], tmp)
                    # cast + transpose back to feature-major attnT.
                    # acc_bf packs both heads per q-slice: [q, j, he, dk],
                    # so each [128, 128] transpose lands as attnT's
                    # [he0 dk rows | he1 dk rows] block directly.
                    acc_bf = accbf.tile([128, 4, 2, 64], BF16, tag="accbf", name="acc_bf")
                    for he in range(2):
                        nc.vector.tensor_copy(acc_bf[:, :, he, :], accs[he])
                    for j in range(4):
                        nc.sync.dma_start_transpose(
                            attnT[:, m, s * SEG + j * 128:s * SEG + (j + 1) * 128],
                            acc_bf[:, j, :, :],
                        )

            def emit_oproj(s):
                for sub in range(SEG // 128):
                    tcn = s * (SEG // 128) + sub
                    y_t = ysb.tile([128, DIM], BF16, tag="ysb", name="y_t")
                    for nh in range(2):
                        ps_y = yps.tile([128, 512], F32, tag="y", name="ps_y")
                        for m in range(2):
                            nc.tensor.matmul(
                                ps_y,
                                attnT[:, m, tcn * 128:(tcn + 1) * 128],
                                wo_sb[:, m, nh * 512:(nh + 1) * 512],
                                start=(m == 0),
                                stop=(m == 1),
                            )
                        if nh == 0:
                            nc.scalar.activation(y_t[:, 0:512], ps_y, Copy)
                        else:
                            nc.vector.tensor_copy(y_t[:, 512:1024], ps_y)
                    # y goes out via SWDGE (idle GpSimd) so it doesn't
                    # queue behind the attnT transposes on the SP ring.
                    nc.gpsimd.dma_start(y[tcn * 128:(tcn + 1) * 128, :], y_t)

            for s in range(S + 1):
                if s < S:
                    emit_attn(s)
                if s >= 1:
                    emit_oproj(s - 1)

    nc.compile()
    return nc


def _make_in_maps(x, Wq, Wk, Wv, Wo):
    bf = ml_dtypes.bfloat16
    xt_b = [np.asarray(x[b]).T.astype(bf) for b in range(B)]
    wq_g = [np.asarray(Wq[:, g * FL:(g + 1) * FL]).astype(bf) for g in range(4)]
    wk_g = [np.asarray(Wk[:, g * FL:(g + 1) * FL]).astype(bf) for g in range(4)]
    wv_g = [np.asarray(Wv[:, g * FL:(g + 1) * FL]).astype(bf) for g in range(4)]
    wo_g = [np.asarray(Wo[g * FL:(g + 1) * FL, :]).astype(bf) for g in range(4)]
    in_maps = []
    for c in range(N_CORES):
        b, g = divmod(c, 4)
        in_maps.append(
            {"xt": xt_b[b], "wq": wq_g[g], "wk": wk_g[g], "wv": wv_g[g],
             "wo": wo_g[g]}
        )
    return in_maps


def run(x, Wq, bq, Wk, bk, Wv, bv, Wo, bo, trace=False, tmpdir=None):
    """Build (cached), run on 8 cores, gather. Returns (y, BassKernelResults)."""
    global _prog
    if _prog is None:
        _prog = _build_program()
    in_maps = _make_in_maps(x, Wq, Wk, Wv, Wo)
    res = run_bass_kernel_spmd(
        _prog, in_maps, core_ids=list(range(N_CORES)), trace=trace, tmpdir=tmpdir
    )
    y = np.zeros((B, L, DIM), np.float32)
    for c in range(N_CORES):
        y[c // 4] += np.asarray(res.results[c]["y"], dtype=np.float32)
    # bq/bk/bv are identically zero in this problem; bo is added on host.
    y += np.asarray(bo, np.float32)[None, None, :]
    return y, res


def kernel(x, Wq, bq, Wk, bk, Wv, bv, Wo, bo):
    y, _ = run(x, Wq, bq, Wk, bk, Wv, bv, Wo, bo)
    return y


# revision 7
# speedup vs baseline: 1.1778x; 1.0022x over previous
"""Dilated self-attention Trainium2 kernel (8-core SPMD), v3.

Problem: x[2, 8192, 1024] -> q/k/v projections -> segment-local dense
attention (SEG=512) + 4 dilated-neighbor cross-attention passes
(offsets +-1, +-2 segments, every 4th key, each with its own softmax,
weight 1/4) -> output projection.

Sharding: data-parallel over batch (2) x tensor-parallel over heads
(4 groups of 4 heads).  Each of the 8 cores runs an IDENTICAL program
on different inputs: xT for its batch, the 256-wide head-group slices
of Wq/Wk/Wv and the matching 256 rows of Wo.  Each core emits a partial
output y[8192, 1024] (bf16); the host sums the 4 head-group partials
per batch.

On-core layout (all matmul operands pre-transposed so no on-device
transposes are needed):
  qT/kT  [128, 2, 8192]  features-on-partitions (head h -> chunk h//2,
                          rows (h%2)*64..)
  v5     [128, 64, 260]  tokens-on-partitions; per head 65 cols
                          [V_h (64) | ones] so each AV matmul also
                          produces the softmax denominator
  vd5    [128, 16, 260]  same for the dilated (every 4th) tokens, with
                          ones=4.0 so the denominator absorbs the
                          1/(2*NN) cross-pass weight.  Gathered from v5
                          by partition-strided SBUF DMA (not recomputed
                          on TensorE).
Scores are computed transposed (S^T[keys, q]); exp runs on ScalarE into
bf16.  AV runs in the [q, dk] orientation (lhsT = exp(S^T) chunk, rhs =
V'): out[q, j*65+64] is the softmax denominator as a per-partition
scalar, so normalization is a [128,4] reciprocal + one broadcast
multiply on the VectorE.  The accumulated [q, dk] result is moved into
the feature-major attnT layout with bf16 DMA transposes.

v3 vs the phase-separated baseline:
  - Attention and output projection are emitted interleaved (one pool
    scope): the attention stretch is ScalarE-exp-bound, so the oproj
    matmuls, PSUM drains and y DMAs hide underneath it.
  - vd5 gathered by DMA (saves a TensorE pass over the dilated tokens).
  - y emitted in bf16 (halves output DMA); host sums in fp32.
"""

import sys

sys.path.insert(0, "/opt/trn_rl_repo")

from contextlib import ExitStack

import numpy as np
import ml_dtypes

import concourse.tile as tile
from concourse import bacc, mybir
from concourse.bass_utils import run_bass_kernel_spmd

BF16 = mybir.dt.bfloat16
F32 = mybir.dt.float32

DIM = 1024
H = 16
DK = 64
SEG = 512
NN = 2
DIL = 4
B = 2
L = 8192
S = L // SEG            # 16 segments
HL = 4                  # heads per core
FL = HL * DK            # 256 features per core
KC = DIM // 128         # 8 contraction chunks for projections
KCS = SEG // 128        # 4 key chunks per segment (local attention)
N_CORES = 8
SCALE = 1.0 / 8.0       # 1/sqrt(DK)

_prog = None


def _build_program():
    nc = bacc.Bacc(None)
    xt = nc.dram_tensor("xt", [DIM, L], BF16, kind="ExternalInput")
    wq = nc.dram_tensor("wq", [DIM, FL], BF16, kind="ExternalInput")
    wk = nc.dram_tensor("wk", [DIM, FL], BF16, kind="ExternalInput")
    wv = nc.dram_tensor("wv", [DIM, FL], BF16, kind="ExternalInput")
    wo = nc.dram_tensor("wo", [FL, DIM], BF16, kind="ExternalInput")
    y = nc.dram_tensor("y", [L, DIM], BF16, kind="ExternalOutput")

    Exp = mybir.ActivationFunctionType.Exp
    Copy = mybir.ActivationFunctionType.Copy

    with tile.TileContext(nc) as tc, ExitStack() as ctx:
        singles = ctx.enter_context(tc.tile_pool(name="singles", bufs=1))
        qt = singles.tile([128, 2, L], BF16)
        kt = singles.tile([128, 2, L], BF16)
        v5 = singles.tile([128, L // 128, HL * 65], BF16)
        vd5 = singles.tile([128, (L // DIL) // 128, HL * 65], BF16)
        attnT = singles.tile([128, 2, L], BF16)
        wq_sb = singles.tile([128, KC, FL], BF16)
        wk_sb = singles.tile([128, KC, FL], BF16)
        wv_sb = singles.tile([128, KC, FL], BF16)
        wo_sb = singles.tile([128, FL // 128, DIM], BF16)
        nc.sync.dma_start(wq_sb, wq.rearrange("(k p) f -> p k f", p=128))
        nc.sync.dma_start(wk_sb, wk.rearrange("(k p) f -> p k f", p=128))
        nc.sync.dma_start(wv_sb, wv.rearrange("(k p) f -> p k f", p=128))
        nc.sync.dma_start(wo_sb, wo.rearrange("(k p) f -> p k f", p=128))
        v5_g = v5.rearrange("p c (h e) -> p c h e", e=65)
        vd5_g = vd5.rearrange("p c (h e) -> p c h e", e=65)
        nc.vector.memset(v5_g[:, :, :, 64], 1.0)
        nc.vector.memset(vd5_g[:, :, :, 64], float(2 * NN))

        # ---------- Phase 1: q/k/v projections (+ dilated-v gather) ----------
        with tc.tile_pool(name="xp", bufs=2) as xp, \
             tc.tile_pool(name="qkps", bufs=4, space="PSUM") as qkps, \
             tc.tile_pool(name="vps", bufs=4, space="PSUM") as vps:
            for t in range(S):
                sl = slice(t * SEG, (t + 1) * SEG)
                x_t = xp.tile([128, KC, SEG], BF16, tag="x", name="x_t")
                nc.sync.dma_start(x_t, xt[:, sl].rearrange("(k p) n -> p k n", p=128))
                for m in range(2):
                    for w_sb, dst in ((wq_sb, qt), (wk_sb, kt)):
                        pst = qkps.tile([128, SEG], F32, tag="qk", name="pst")
                        for k in range(KC):
                            nc.tensor.matmul(
                                pst,
                                w_sb[:, k, m * 128:(m + 1) * 128],
                                x_t[:, k],
                                start=(k == 0),
                                stop=(k == KC - 1),
                            )
                        eng = nc.vector if m == 0 else nc.scalar
                        if m == 0:
                            nc.vector.tensor_copy(dst[:, m, sl], pst)
                        else:
                            nc.scalar.activation(dst[:, m, sl], pst, Copy)
                for sub in range(SEG // 128):
                    c = t * (SEG // 128) + sub
                    psv = vps.tile([128, FL], F32, tag="v", name="psv")
                    for k in range(KC):
                        nc.tensor.matmul(
                            psv,
                            x_t[:, k, sub * 128:(sub + 1) * 128],
                            wv_sb[:, k],
                            start=(k == 0),
                            stop=(k == KC - 1),
                        )
                    if sub % 2 == 0:
                        nc.scalar.activation(
                            v5_g[:, c, :, 0:64],
                            psv.rearrange("p (h e) -> p h e", e=64),
                            Copy,
                        )
                    else:
                        nc.vector.tensor_copy(
                            v5_g[:, c, :, 0:64],
                            psv.rearrange("p (h e) -> p h e", e=64),
                        )
                # dilated V: gather every 4th token of this segment from
                # v5 (partition-strided SBUF->SBUF DMA, replaces a
                # TensorE projection pass over the dilated tokens).
                for a in range(4):
                    nc.sync.dma_start(
                        vd5_g[32 * a:32 * (a + 1), t, :, 0:64],
                        v5_g[0:128:DIL, t * 4 + a, :, 0:64],
                    )

        # ---------- Phase 2+3 fused: attention + output projection ----------
        # The attention stream is ScalarE(exp)-bound; interleaving the
        # output-projection matmuls/drains/DMAs per segment fills the
        # TensorE/VectorE idle time underneath it.
        with tc.tile_pool(name="scps", bufs=2, space="PSUM") as scps, \
             tc.tile_pool(name="avl", bufs=1, space="PSUM") as avl, \
             tc.tile_pool(name="avx", bufs=2, space="PSUM") as avx, \
             tc.tile_pool(name="yps", bufs=1, space="PSUM") as yps, \
             tc.tile_pool(name="expp", bufs=6) as expp, \
             tc.tile_pool(name="recp", bufs=6) as recp, \
             tc.tile_pool(name="accp", bufs=6) as accp, \
             tc.tile_pool(name="accbf", bufs=4) as accbf, \
             tc.tile_pool(name="ysb", bufs=3) as ysb:

            def emit_attn(s):
                q_sl = slice(s * SEG, (s + 1) * SEG)
                for m in range(2):
                    # local scores S^T = K^T-chunks x Q (two heads as
                    # concurrent row-tiles at partition bases 0 / 64)
                    exp_tiles = []
                    for c in range(KCS):
                        k_sl = slice(s * SEG + c * 128, s * SEG + (c + 1) * 128)
                        ps_sc = scps.tile([128, 2 * SEG], F32, tag="sc", name="ps_sc")
                        for he in range(2):
                            r0 = he * 64
                            nc.tensor.matmul(
                                ps_sc[:, he * SEG:(he + 1) * SEG],
                                kt[r0:r0 + 64, m, k_sl],
                                qt[r0:r0 + 64, m, q_sl],
                            )
                        e_t = expp.tile([128, 2 * SEG], BF16, tag="exp", name="e_t")
                        nc.scalar.activation(e_t, ps_sc, Exp, scale=SCALE)
                        exp_tiles.append(e_t)
                    # local AV in [q, dk] orientation; col j*65+64 is the
                    # softmax denominator (per-partition scalar)
                    accs = []
                    for he in range(2):
                        hl = 2 * m + he
                        av = avl.tile([128, 260], F32, tag="avl", name="av")
                        for c in range(KCS):
                            for j in range(4):
                                nc.tensor.matmul(
                                    av[:, j * 65:(j + 1) * 65],
                                    exp_tiles[c][:, he * SEG + j * 128:he * SEG + (j + 1) * 128],
                                    v5[:, s * KCS + c, hl * 65:(hl + 1) * 65],
                                    start=(c == 0 and j == 0),
                                    stop=(c == KCS - 1 and j == 3),
                                )
                        av_g = av.rearrange("p (j e) -> p j e", e=65)
                        rec = recp.tile([128, 4], F32, tag="rec", name="rec")
                        nc.vector.reciprocal(rec, av_g[:, :, 64])
                        acc = accp.tile([128, 4, 64], F32, tag="acc", name="acc")
                        nc.vector.tensor_mul(
                            acc, av_g[:, :, 0:64],
                            rec[:, :, None].to_broadcast((128, 4, 64)),
                        )
                        accs.append(acc)
                    # dilated neighbor-segment passes
                    valid_offs = [o for o in (-2, -1, 1, 2) if 0 <= s + o < S]
                    for idx, o in enumerate(valid_offs):
                        n = s + o
                        ps_sc = scps.tile([128, 2 * SEG], F32, tag="sc", name="ps_sc")
                        for he in range(2):
                            r0 = he * 64
                            nc.tensor.matmul(
                                ps_sc[:, he * SEG:(he + 1) * SEG],
                                kt[r0:r0 + 64, m, n * SEG:(n + 1) * SEG:DIL],
                                qt[r0:r0 + 64, m, q_sl],
                            )
                        e_t = expp.tile([128, 2 * SEG], BF16, tag="exp", name="e_t")
                        nc.scalar.activation(e_t, ps_sc, Exp, scale=SCALE)
                        for he in range(2):
                            hl = 2 * m + he
                            avx_t = avx.tile([128, 260], F32, tag="avx", name="avx_t")
                            for j in range(4):
                                nc.tensor.matmul(
                                    avx_t[:, j * 65:(j + 1) * 65],
                                    e_t[:, he * SEG + j * 128:he * SEG + (j + 1) * 128],
                                    vd5[:, n, hl * 65:(hl + 1) * 65],
                                    start=(j == 0),
                                    stop=(j == 3),
                                )
                            avx_g = avx_t.rearrange("p (j e) -> p j e", e=65)
                            rec = recp.tile([128, 4], F32, tag="rec", name="rec")
                            nc.vector.reciprocal(rec, avx_g[:, :, 64])
                            tmp = accp.tile([128, 4, 64], F32, tag="tmp", name="tmp")
                            nc.vector.tensor_mul(
                                tmp, avx_g[:, :, 0:64],
                                rec[:, :, None].to_broadcast((128, 4, 64)),
                            )
                            nc.gpsimd.tensor_add(accs[he], accs[he], tmp)
                    # cast + transpose back to feature-major attnT.
                    # acc_bf packs both heads per q-slice: [q, j, he, dk],
                    # so each [128, 128] transpose lands as attnT's
                    # [he0 dk rows | he1 dk rows] block directly.
                    acc_bf = accbf.tile([128, 4, 2, 64], BF16, tag="accbf", name="acc_bf")
                    for he in range(2):
                        nc.gpsimd.tensor_copy(acc_bf[:, :, he, :], accs[he])
                    for j in range(4):
                        nc.sync.dma_start_transpose(
                            attnT[:, m, s * SEG + j * 128:s * SEG + (j + 1) * 128],
                            acc_bf[:, j, :, :],
                        )

            def emit_oproj(s):
                for sub in range(SEG // 128):
                    tcn = s * (SEG // 128) + sub
                    y_t = ysb.tile([128, DIM], BF16, tag="ysb", name="y_t")
                    for nh in range(2):
                        ps_y = yps.tile([128, 512], F32, tag="y", name="ps_y")
                        for m in range(2):
                            nc.tensor.matmul(
                                ps_y,
                                attnT[:, m, tcn * 128:(tcn + 1) * 128],
                                wo_sb[:, m, nh * 512:(nh + 1) * 512],
                                start=(m == 0),
                                stop=(m == 1),
                            )
                        nc.vector.tensor_copy(y_t[:, nh * 512:(nh + 1) * 512], ps_y)
                    nc.sync.dma_start(y[tcn * 128:(tcn + 1) * 128, :], y_t)

            for s in range(S + 1):
                if s < S:
                    emit_attn(s)
                if s >= 1:
                    emit_oproj(s - 1)

    nc.compile()
    return nc


def _make_in_maps(x, Wq, Wk, Wv, Wo):
    bf = ml_dtypes.bfloat16
    xt_b = [np.asarray(x[b]).T.astype(bf) for b in range(B)]
    wq_g = [np.asarray(Wq[:, g * FL:(g + 1) * FL]).astype(bf) for g in range(4)]
    wk_g = [np.asarray(Wk[:, g * FL:(g + 1) * FL]).astype(bf) for g in range(4)]
    wv_g = [np.asarray(Wv[:, g * FL:(g + 1) * FL]).astype(bf) for g in range(4)]
    wo_g = [np.asarray(Wo[g * FL:(g + 1) * FL, :]).astype(bf) for g in range(4)]
    in_maps = []
    for c in range(N_CORES):
        b, g = divmod(c, 4)
        in_maps.append(
            {"xt": xt_b[b], "wq": wq_g[g], "wk": wk_g[g], "wv": wv_g[g],
             "wo": wo_g[g]}
        )
    return in_maps


def run(x, Wq, bq, Wk, bk, Wv, bv, Wo, bo, trace=False, tmpdir=None):
    """Build (cached), run on 8 cores, gather. Returns (y, BassKernelResults)."""
    global _prog
    if _prog is None:
        _prog = _build_program()
    in_maps = _make_in_maps(x, Wq, Wk, Wv, Wo)
    res = run_bass_kernel_spmd(
        _prog, in_maps, core_ids=list(range(N_CORES)), trace=trace, tmpdir=tmpdir
    )
    y = np.zeros((B, L, DIM), np.float32)
    for c in range(N_CORES):
        y[c // 4] += np.asarray(res.results[c]["y"], dtype=np.float32)
    # bq/bk/bv are identically zero in this problem; bo is added on host.
    y += np.asarray(bo, np.float32)[None, None, :]
    return y, res


def kernel(x, Wq, bq, Wk, bk, Wv, bv, Wo, bo):
    y, _ = run(x, Wq, bq, Wk, bk, Wv, bv, Wo, bo)
    return y


# revision 9
# speedup vs baseline: 1.2106x; 1.0278x over previous
"""Dilated self-attention Trainium2 kernel (8-core SPMD), v3.

Problem: x[2, 8192, 1024] -> q/k/v projections -> segment-local dense
attention (SEG=512) + 4 dilated-neighbor cross-attention passes
(offsets +-1, +-2 segments, every 4th key, each with its own softmax,
weight 1/4) -> output projection.

Sharding: data-parallel over batch (2) x tensor-parallel over heads
(4 groups of 4 heads).  Each of the 8 cores runs an IDENTICAL program
on different inputs: xT for its batch, the 256-wide head-group slices
of Wq/Wk/Wv and the matching 256 rows of Wo.  Each core emits a partial
output y[8192, 1024] (bf16); the host sums the 4 head-group partials
per batch.

On-core layout (all matmul operands pre-transposed so no on-device
transposes are needed):
  qT/kT  [128, 2, 8192]  features-on-partitions (head h -> chunk h//2,
                          rows (h%2)*64..)
  v5     [128, 64, 260]  tokens-on-partitions; per head 65 cols
                          [V_h (64) | ones] so each AV matmul also
                          produces the softmax denominator
  vd5    [128, 16, 260]  same for the dilated (every 4th) tokens, with
                          ones=4.0 so the denominator absorbs the
                          1/(2*NN) cross-pass weight.  Gathered from v5
                          by partition-strided SBUF DMA (not recomputed
                          on TensorE).
Scores are computed transposed (S^T[keys, q]); exp runs on ScalarE into
bf16.  AV runs in the [q, dk] orientation (lhsT = exp(S^T) chunk, rhs =
V'): out[q, j*65+64] is the softmax denominator as a per-partition
scalar, so normalization is a [128,4] reciprocal + one broadcast
multiply on the VectorE.  The accumulated [q, dk] result is moved into
the feature-major attnT layout with bf16 DMA transposes.

v3 vs the phase-separated baseline:
  - Attention and output projection are emitted interleaved (one pool
    scope): the attention stretch is ScalarE-exp-bound, so the oproj
    matmuls, PSUM drains and y DMAs hide underneath it.
  - vd5 gathered by DMA (saves a TensorE pass over the dilated tokens).
  - y emitted in bf16 (halves output DMA); host sums in fp32.
"""

import sys

sys.path.insert(0, "/opt/trn_rl_repo")

from contextlib import ExitStack

import numpy as np
import ml_dtypes

import concourse.tile as tile
from concourse import bacc, mybir
from concourse.bass_utils import run_bass_kernel_spmd

BF16 = mybir.dt.bfloat16
F32 = mybir.dt.float32

DIM = 1024
H = 16
DK = 64
SEG = 512
NN = 2
DIL = 4
B = 2
L = 8192
S = L // SEG            # 16 segments
HL = 4                  # heads per core
FL = HL * DK            # 256 features per core
KC = DIM // 128         # 8 contraction chunks for projections
KCS = SEG // 128        # 4 key chunks per segment (local attention)
N_CORES = 8
SCALE = 1.0 / 8.0       # 1/sqrt(DK)

_prog = None


def _build_program():
    nc = bacc.Bacc(None)
    xt = nc.dram_tensor("xt", [DIM, L], BF16, kind="ExternalInput")
    wq = nc.dram_tensor("wq", [DIM, FL], BF16, kind="ExternalInput")
    wk = nc.dram_tensor("wk", [DIM, FL], BF16, kind="ExternalInput")
    wv = nc.dram_tensor("wv", [DIM, FL], BF16, kind="ExternalInput")
    wo = nc.dram_tensor("wo", [FL, DIM], BF16, kind="ExternalInput")
    y = nc.dram_tensor("y", [L, DIM], BF16, kind="ExternalOutput")

    Exp = mybir.ActivationFunctionType.Exp
    Copy = mybir.ActivationFunctionType.Copy

    with tile.TileContext(nc) as tc, ExitStack() as ctx:
        singles = ctx.enter_context(tc.tile_pool(name="singles", bufs=1))
        qt = singles.tile([128, 2, L], BF16)
        kt = singles.tile([128, 2, L], BF16)
        v5 = singles.tile([128, L // 128, HL * 65], BF16)
        vd5 = singles.tile([128, (L // DIL) // 128, HL * 65], BF16)
        attnT = singles.tile([128, 2, L], BF16)
        wq_sb = singles.tile([128, KC, FL], BF16)
        wk_sb = singles.tile([128, KC, FL], BF16)
        wv_sb = singles.tile([128, KC, FL], BF16)
        wo_sb = singles.tile([128, FL // 128, DIM], BF16)
        nc.sync.dma_start(wq_sb, wq.rearrange("(k p) f -> p k f", p=128))
        nc.sync.dma_start(wk_sb, wk.rearrange("(k p) f -> p k f", p=128))
        nc.sync.dma_start(wv_sb, wv.rearrange("(k p) f -> p k f", p=128))
        nc.sync.dma_start(wo_sb, wo.rearrange("(k p) f -> p k f", p=128))
        v5_g = v5.rearrange("p c (h e) -> p c h e", e=65)
        vd5_g = vd5.rearrange("p c (h e) -> p c h e", e=65)
        nc.vector.memset(v5_g[:, :, :, 64], 1.0)
        nc.vector.memset(vd5_g[:, :, :, 64], float(2 * NN))

        # ---------- Phase 1: q/k/v projections (+ dilated-v gather) ----------
        with tc.tile_pool(name="xp", bufs=2) as xp, \
             tc.tile_pool(name="qkps", bufs=4, space="PSUM") as qkps, \
             tc.tile_pool(name="vps", bufs=4, space="PSUM") as vps:
            for t in range(S):
                sl = slice(t * SEG, (t + 1) * SEG)
                x_t = xp.tile([128, KC, SEG], BF16, tag="x", name="x_t")
                nc.sync.dma_start(x_t, xt[:, sl].rearrange("(k p) n -> p k n", p=128))
                for m in range(2):
                    for w_sb, dst in ((wq_sb, qt), (wk_sb, kt)):
                        pst = qkps.tile([128, SEG], F32, tag="qk", name="pst")
                        for k in range(KC):
                            nc.tensor.matmul(
                                pst,
                                w_sb[:, k, m * 128:(m + 1) * 128],
                                x_t[:, k],
                                start=(k == 0),
                                stop=(k == KC - 1),
                            )
                        eng = nc.vector if m == 0 else nc.scalar
                        if m == 0:
                            nc.vector.tensor_copy(dst[:, m, sl], pst)
                        else:
                            nc.scalar.activation(dst[:, m, sl], pst, Copy)
                for sub in range(SEG // 128):
                    c = t * (SEG // 128) + sub
                    psv = vps.tile([128, FL], F32, tag="v", name="psv")
                    for k in range(KC):
                        nc.tensor.matmul(
                            psv,
                            x_t[:, k, sub * 128:(sub + 1) * 128],
                            wv_sb[:, k],
                            start=(k == 0),
                            stop=(k == KC - 1),
                        )
                    if sub % 2 == 0:
                        nc.scalar.activation(
                            v5_g[:, c, :, 0:64],
                            psv.rearrange("p (h e) -> p h e", e=64),
                            Copy,
                        )
                    else:
                        nc.vector.tensor_copy(
                            v5_g[:, c, :, 0:64],
                            psv.rearrange("p (h e) -> p h e", e=64),
                        )
                # dilated V: gather every 4th token of this segment from
                # v5 (partition-strided SBUF->SBUF DMA, replaces a
                # TensorE projection pass over the dilated tokens).
                for a in range(4):
                    nc.sync.dma_start(
                        vd5_g[32 * a:32 * (a + 1), t, :, 0:64],
                        v5_g[0:128:DIL, t * 4 + a, :, 0:64],
                    )

        # ---------- Phase 2+3 fused: attention + output projection ----------
        # The attention stream is ScalarE(exp)-bound; interleaving the
        # output-projection matmuls/drains/DMAs per segment fills the
        # TensorE/VectorE idle time underneath it.
        with tc.tile_pool(name="scps", bufs=2, space="PSUM") as scps, \
             tc.tile_pool(name="avl", bufs=1, space="PSUM") as avl, \
             tc.tile_pool(name="avx", bufs=2, space="PSUM") as avx, \
             tc.tile_pool(name="yps", bufs=1, space="PSUM") as yps, \
             tc.tile_pool(name="expp", bufs=6) as expp, \
             tc.tile_pool(name="recp", bufs=6) as recp, \
             tc.tile_pool(name="accp", bufs=6) as accp, \
             tc.tile_pool(name="accbf", bufs=4) as accbf, \
             tc.tile_pool(name="ysb", bufs=3) as ysb:

            def emit_attn(s):
                q_sl = slice(s * SEG, (s + 1) * SEG)
                for m in range(2):
                    # local scores S^T = K^T-chunks x Q (two heads as
                    # concurrent row-tiles at partition bases 0 / 64)
                    exp_tiles = []
                    for c in range(KCS):
                        k_sl = slice(s * SEG + c * 128, s * SEG + (c + 1) * 128)
                        ps_sc = scps.tile([128, 2 * SEG], F32, tag="sc", name="ps_sc")
                        for he in range(2):
                            r0 = he * 64
                            nc.tensor.matmul(
                                ps_sc[:, he * SEG:(he + 1) * SEG],
                                kt[r0:r0 + 64, m, k_sl],
                                qt[r0:r0 + 64, m, q_sl],
                            )
                        e_t = expp.tile([128, 2 * SEG], BF16, tag="exp", name="e_t")
                        nc.scalar.activation(e_t, ps_sc, Exp, scale=SCALE)
                        exp_tiles.append(e_t)
                    # local AV in [q, dk] orientation; col j*65+64 is the
                    # softmax denominator (per-partition scalar)
                    accs = []
                    for he in range(2):
                        hl = 2 * m + he
                        av = avl.tile([128, 260], F32, tag="avl", name="av")
                        for c in range(KCS):
                            for j in range(4):
                                for half in range(2):
                                    base = he * SEG + j * 128 + half * 64
                                    nc.tensor.matmul(
                                        av[half * 64:(half + 1) * 64, j * 65:(j + 1) * 65],
                                        exp_tiles[c][:, base:base + 64],
                                        v5[:, s * KCS + c, hl * 65:(hl + 1) * 65],
                                        start=(c == 0 and j == 0),
                                        stop=(c == KCS - 1 and j == 3),
                                        tile_position=(0, half * 64),
                                    )
                        av_g = av.rearrange("p (j e) -> p j e", e=65)
                        rec = recp.tile([128, 4], F32, tag="rec", name="rec")
                        nc.vector.reciprocal(rec, av_g[:, :, 64])
                        acc = accp.tile([128, 4, 64], F32, tag="acc", name="acc")
                        nc.vector.tensor_mul(
                            acc, av_g[:, :, 0:64],
                            rec[:, :, None].to_broadcast((128, 4, 64)),
                        )
                        accs.append(acc)
                    # dilated neighbor-segment passes
                    valid_offs = [o for o in (-2, -1, 1, 2) if 0 <= s + o < S]
                    for idx, o in enumerate(valid_offs):
                        n = s + o
                        ps_sc = scps.tile([128, 2 * SEG], F32, tag="sc", name="ps_sc")
                        for he in range(2):
                            r0 = he * 64
                            nc.tensor.matmul(
                                ps_sc[:, he * SEG:(he + 1) * SEG],
                                kt[r0:r0 + 64, m, n * SEG:(n + 1) * SEG:DIL],
                                qt[r0:r0 + 64, m, q_sl],
                            )
                        e_t = expp.tile([128, 2 * SEG], BF16, tag="exp", name="e_t")
                        nc.scalar.activation(e_t, ps_sc, Exp, scale=SCALE)
                        for he in range(2):
                            hl = 2 * m + he
                            avx_t = avx.tile([128, 260], F32, tag="avx", name="avx_t")
                            for j in range(4):
                                for half in range(2):
                                    base = he * SEG + j * 128 + half * 64
                                    nc.tensor.matmul(
                                        avx_t[half * 64:(half + 1) * 64, j * 65:(j + 1) * 65],
                                        e_t[:, base:base + 64],
                                        vd5[:, n, hl * 65:(hl + 1) * 65],
                                        start=(j == 0),
                                        stop=(j == 3),
                                        tile_position=(0, half * 64),
                                    )
                            avx_g = avx_t.rearrange("p (j e) -> p j e", e=65)
                            rec = recp.tile([128, 4], F32, tag="rec", name="rec")
                            nc.vector.reciprocal(rec, avx_g[:, :, 64])
                            tmp = accp.tile([128, 4, 64], F32, tag="tmp", name="tmp")
                            nc.vector.tensor_mul(
                                tmp, avx_g[:, :, 0:64],
                                rec[:, :, None].to_broadcast((128, 4, 64)),
                            )
                            nc.vector.tensor_add(accs[he], accs[he], tmp)
                    # cast + transpose back to feature-major attnT.
                    # acc_bf packs both heads per q-slice: [q, j, he, dk],
                    # so each [128, 128] transpose lands as attnT's
                    # [he0 dk rows | he1 dk rows] block directly.
                    acc_bf = accbf.tile([128, 4, 2, 64], BF16, tag="accbf", name="acc_bf")
                    for he in range(2):
                        nc.vector.tensor_copy(acc_bf[:, :, he, :], accs[he])
                    for j in range(4):
                        nc.sync.dma_start_transpose(
                            attnT[:, m, s * SEG + j * 128:s * SEG + (j + 1) * 128],
                            acc_bf[:, j, :, :],
                        )

            def emit_oproj(s):
                for sub in range(SEG // 128):
                    tcn = s * (SEG // 128) + sub
                    y_t = ysb.tile([128, DIM], BF16, tag="ysb", name="y_t")
                    for nh in range(2):
                        ps_y = yps.tile([128, 512], F32, tag="y", name="ps_y")
                        for m in range(2):
                            nc.tensor.matmul(
                                ps_y,
                                attnT[:, m, tcn * 128:(tcn + 1) * 128],
                                wo_sb[:, m, nh * 512:(nh + 1) * 512],
                                start=(m == 0),
                                stop=(m == 1),
                            )
                        if nh == 0:
                            nc.scalar.activation(y_t[:, 0:512], ps_y, Copy)
                        else:
                            nc.vector.tensor_copy(y_t[:, 512:1024], ps_y)
                    nc.sync.dma_start(y[tcn * 128:(tcn + 1) * 128, :], y_t)

            for s in range(S + 1):
                if s < S:
                    emit_attn(s)
                if s >= 1:
                    emit_oproj(s - 1)

    nc.compile()
    return nc


def _make_in_maps(x, Wq, Wk, Wv, Wo):
    bf = ml_dtypes.bfloat16
    xt_b = [np.asarray(x[b]).T.astype(bf) for b in range(B)]
    wq_g = [np.asarray(Wq[:, g * FL:(g + 1) * FL]).astype(bf) for g in range(4)]
    wk_g = [np.asarray(Wk[:, g * FL:(g + 1) * FL]).astype(bf) for g in range(4)]
    wv_g = [np.asarray(Wv[:, g * FL:(g + 1) * FL]).astype(bf) for g in range(4)]
    wo_g = [np.asarray(Wo[g * FL:(g + 1) * FL, :]).astype(bf) for g in range(4)]
    in_maps = []
    for c in range(N_CORES):
        b, g = divmod(c, 4)
        in_maps.append(
            {"xt": xt_b[b], "wq": wq_g[g], "wk": wk_g[g], "wv": wv_g[g],
             "wo": wo_g[g]}
        )
    return in_maps


def run(x, Wq, bq, Wk, bk, Wv, bv, Wo, bo, trace=False, tmpdir=None):
    """Build (cached), run on 8 cores, gather. Returns (y, BassKernelResults)."""
    global _prog
    if _prog is None:
        _prog = _build_program()
    in_maps = _make_in_maps(x, Wq, Wk, Wv, Wo)
    res = run_bass_kernel_spmd(
        _prog, in_maps, core_ids=list(range(N_CORES)), trace=trace, tmpdir=tmpdir
    )
    y = np.zeros((B, L, DIM), np.float32)
    for c in range(N_CORES):
        y[c // 4] += np.asarray(res.results[c]["y"], dtype=np.float32)
    # bq/bk/bv are identically zero in this problem; bo is added on host.
    y += np.asarray(bo, np.float32)[None, None, :]
    return y, res


def kernel(x, Wq, bq, Wk, bk, Wv, bv, Wo, bo):
    y, _ = run(x, Wq, bq, Wk, bk, Wv, bv, Wo, bo)
    return y
